# revision 3
# baseline (speedup 1.0000x reference)
"""Trainium2 Bass kernel for nn_DenoiseModule (diffraction removal + 2x2 Wiener).

Math reduction (derived from the reference):
  - The reference FFT2 acts on the (W, C) axes of the (B,H,W,C)-transposed
    image. The Gaussian mask factorizes mask[h,w] = a[h]*s[w] and is constant
    along the channel-frequency axis, so the C-axis FFT cancels exactly.
  - Net effect: per (b,c,h) row, a 1-D circular convolution along W with the
    short kernel K = IFFT(s), scaled by a[h], then abs(), then the
    scipy-style 2x2 Wiener filter.
  - K's imaginary part contributes < 1e-5 rel error to the final output
    (it exists only because linspace(-256,256,512) is slightly asymmetric),
    so only the real part is convolved and |y| = abs(real conv).
  - The x255/255 scaling cancels; a[h] commutes with the W-conv so it is
    applied to the input on the host.

Device layout: batch-parallel over 8 cores (4 images each = 12 channels).
Per channel, data is W-on-partitions (host pre-transposes). Everything is
fp16 on device (the harness tolerance is 2e-2; fp16 keeps rel err ~1e-3):
  - conv: 2 fp16 matmuls per 128-row chunk (main band block + one 16-row
    halo matmul covering both cross-chunk edge triangles)
  - box sums: h-pairs pre-added on DVE (4x-mode STT ops), then 2 matmuls
    per chunk per box (in-chunk band + cross-chunk edge)
  - Wiener tail: 4x/2x DVE ops; the elementwise divide runs on GPSIMD.
"""
import numpy as np

B, C, H, W = 32, 3, 512, 512
NCORES = 8
BL = B // NCORES          # images per core
NCH = BL * C              # channels per core
P = 128
NW = W // P               # w-chunks
TAP = 8                   # conv band half-width
HALO = 16                 # halo rows per chunk (8 below + 8 above)
DR = 40.0


def _constants():
    x_lin = np.linspace(-256, 256, 512).astype(np.float64)
    g = np.exp(-(x_lin ** 2) / (2 * DR * DR))
    sh = (np.arange(512) + 256) % 512
    a = g[sh]                      # per-h scale (fft-order coords)
    s = g[sh]                      # per-kw mask
    K = np.fft.ifft(s)
    d = np.minimum(np.arange(512), 512 - np.arange(512))
    Kr = np.where(d <= TAP, np.real(K), 0.0)
    # main conv block (same for every chunk): M[p, c] = Kr[c - p], |c-p|<=TAP
    pp = np.arange(P)[:, None]
    cc = np.arange(P)[None, :]
    M = np.where(np.abs(cc - pp) <= TAP, Kr[(cc - pp) % 512], 0.0)
    # halo block: rows 0..7 are inputs at chunk_start-8+r, rows 8..15 are
    # inputs at chunk_start+128+(r-8); E[r, c] = Kr[c - src(r)]
    src = np.concatenate([np.arange(-8, 0), np.arange(128, 136)])
    rr_ = src[:, None]
    E = np.where(np.abs(cc - rr_) <= TAP, Kr[(cc - rr_) % 512], 0.0)
    # box lhsT: out[wo] = 0.25*(in[wo] + in[wo-1])
    bx = np.zeros((P, P))
    np.fill_diagonal(bx, 0.25)
    bx[np.arange(P - 1), np.arange(1, P)] = 0.25
    # edge block: out[0] += 0.25*in_prev[127]
    bxe = np.zeros((P, P))
    bxe[P - 1, 0] = 0.25
    f16 = np.float16
    return (a, M.astype(f16), E.astype(f16), bx.astype(f16), bxe.astype(f16))


_PROG_CACHE = {}


def _build_program():
    from contextlib import ExitStack
    import concourse.bacc as bacc
    import concourse.tile as tile
    from concourse import mybir

    f32 = mybir.dt.float32
    f16 = mybir.dt.float16
    Alu = mybir.AluOpType
    Act = mybir.ActivationFunctionType

    nc = bacc.Bacc(None)
    x_in = nc.declare_dram_parameter("x", [NCH, W, H], f16, isOutput=False)
    xh_in = nc.declare_dram_parameter("xh", [NCH, HALO, NW, H], f16, isOutput=False)
    cm_in = nc.declare_dram_parameter("cm", [P, P], f16, isOutput=False)
    ce_in = nc.declare_dram_parameter("ce", [HALO, P], f16, isOutput=False)
    bx_in = nc.declare_dram_parameter("bx", [P, P], f16, isOutput=False)
    bxe_in = nc.declare_dram_parameter("bxe", [P, P], f16, isOutput=False)
    ones_in = nc.declare_dram_parameter("ones", [P, 1], f32, isOutput=False)
    onesr_in = nc.declare_dram_parameter("onesr", [1, P], f32, isOutput=False)
    y_out = nc.declare_dram_parameter("y", [NCH, W, H], f16, isOutput=True)

    HP = H + 2

    with tile.TileContext(nc) as tc, ExitStack() as ctx:
        cpool = ctx.enter_context(tc.tile_pool(name="consts", bufs=1))
        cm_t = cpool.tile([P, P], f16, tag="cm")
        nc.sync.dma_start(cm_t[:], cm_in[:])
        ce_t = cpool.tile([HALO, P], f16, tag="ce")
        nc.sync.dma_start(ce_t[:], ce_in[:])
        bx_t = cpool.tile([P, P], f16, tag="bx")
        nc.sync.dma_start(bx_t[:], bx_in[:])
        bxe_t = cpool.tile([P, P], f16, tag="bxe")
        nc.sync.dma_start(bxe_t[:], bxe_in[:])
        ones_t = cpool.tile([P, 1], f32, tag="ones")
        nc.sync.dma_start(ones_t[:], ones_in[:])
        onesr_t = cpool.tile([1, P], f32, tag="onesr")
        nc.sync.dma_start(onesr_t[:], onesr_in[:])

        xpool = ctx.enter_context(tc.tile_pool(name="xin", bufs=3))
        hpool = ctx.enter_context(tc.tile_pool(name="halo", bufs=3))
        mpool = ctx.enter_context(tc.tile_pool(name="magp", bufs=3))
        spool = ctx.enter_context(tc.tile_pool(name="sqp", bufs=3))
        upool = ctx.enter_context(tc.tile_pool(name="up", bufs=3))
        lpool = ctx.enter_context(tc.tile_pool(name="lvp", bufs=2))
        tpool = ctx.enter_context(tc.tile_pool(name="tmp", bufs=3))
        npool = ctx.enter_context(tc.tile_pool(name="noise", bufs=3))
        opool = ctx.enter_context(tc.tile_pool(name="outp", bufs=2))
        psum = ctx.enter_context(tc.tile_pool(name="ps", bufs=2, space="PSUM"))

        def emit_pass_a(ch):
            st = {"ch": ch}
            xin = xpool.tile([P, NW, H], f16, tag="xin")
            nc.sync.dma_start(
                xin[:], x_in[ch].rearrange("(j p) h -> p j h", p=P)
            )
            halo = hpool.tile([HALO, NW, H], f16, tag="halo")
            nc.sync.dma_start(halo[:], xh_in[ch])

            # mag/sq keep 2 leading zero cols per chunk: data at [2:H+2);
            # the h-1 shifted read [1:H+1) then sees a zero at h=0.
            mag = mpool.tile([P, NW, HP], f16, tag="mag")
            nc.vector.memset(mag[:, :, 0:2], 0.0)
            sq = spool.tile([P, NW, HP], f16, tag="sq")
            nc.vector.memset(sq[:, :, 0:2], 0.0)
            lvar = lpool.tile([P, NW, H], f16, tag="lvar")
            diff = lpool.tile([P, NW, H], f16, tag="diff")
            part = tpool.tile([P, NW], f32, tag="part")
            um_prev = None
            us_prev = None

            for i in range(NW):
                ps_y = psum.tile([P, H], f32, tag="ps_y")
                nc.tensor.matmul(ps_y[:], cm_t[:], xin[:, i, :],
                                 start=True, stop=False)
                nc.tensor.matmul(ps_y[:], ce_t[:], halo[:, i, :],
                                 start=False, stop=True)
                nc.scalar.activation(mag[:, i, 2:H + 2], ps_y[:], Act.Abs)
                # sq = mag^2 (4x-mode STT: (mag*1) mult mag)
                nc.vector.scalar_tensor_tensor(
                    sq[:, i, 2:H + 2], in0=mag[:, i, 2:H + 2], scalar=1.0,
                    in1=mag[:, i, 2:H + 2], op0=Alu.mult, op1=Alu.mult,
                )
                # h-pair pre-adds (4x STT)
                um = upool.tile([P, H], f16, tag="um")
                nc.vector.scalar_tensor_tensor(
                    um[:], in0=mag[:, i, 2:H + 2], scalar=1.0,
                    in1=mag[:, i, 1:H + 1], op0=Alu.mult, op1=Alu.add,
                )
                us = upool.tile([P, H], f16, tag="us")
                nc.vector.scalar_tensor_tensor(
                    us[:], in0=sq[:, i, 2:H + 2], scalar=1.0,
                    in1=sq[:, i, 1:H + 1], op0=Alu.mult, op1=Alu.add,
                )
                # box sums on PE: ps_lm = box(mag)/4, ps_bs = box(sq)/4
                ps_lm = psum.tile([P, H], f32, tag="ps_lm")
                nc.tensor.matmul(ps_lm[:], bx_t[:], um[:],
                                 start=True, stop=(i == 0))
                if i > 0:
                    nc.tensor.matmul(ps_lm[:], bxe_t[:], um_prev[:],
                                     start=False, stop=True)
                ps_bs = psum.tile([P, H], f32, tag="ps_bs")
                nc.tensor.matmul(ps_bs[:], bx_t[:], us[:],
                                 start=True, stop=(i == 0))
                if i > 0:
                    nc.tensor.matmul(ps_bs[:], bxe_t[:], us_prev[:],
                                     start=False, stop=True)
                um_prev, us_prev = um, us

                lm2 = tpool.tile([P, H], f32, tag="lm2")
                nc.scalar.square(lm2[:], ps_lm[:])
                # lvar = ps_bs - lm2 ; accum -> per-partition rowsum for noise
                nc.vector.scalar_tensor_tensor(
                    lvar[:, i, :], in0=lm2[:], scalar=-1.0, in1=ps_bs[:],
                    op0=Alu.mult, op1=Alu.add, accum_out=part[:, i:i + 1],
                )
                # diff = lM - mag
                nc.vector.tensor_tensor(diff[:, i, :], ps_lm[:],
                                        mag[:, i, 2:H + 2], Alu.subtract)

            # ---- noise scalar (PE reduce + broadcast) ----
            pr = tpool.tile([P, 1], f32, tag="pr")
            nc.vector.tensor_reduce(pr[:], part[:], mybir.AxisListType.X, Alu.add)
            ps_n1 = psum.tile([P, H], f32, tag="ps_y")
            nc.tensor.matmul(ps_n1[:1, :1], ones_t[:], pr[:], start=True, stop=True)
            nb = npool.tile([1, 1], f32, tag="nb")
            nc.scalar.copy(nb[:], ps_n1[:1, :1])
            ps_n2 = psum.tile([P, H], f32, tag="ps_lm")
            nc.tensor.matmul(ps_n2[:, :1], onesr_t[:], nb[:],
                             start=True, stop=True)
            noise = npool.tile([P, 1], f32, tag="noise")
            nc.scalar.mul(noise[:], ps_n2[:, :1], 1.0 / (H * W))
            inv_noise = npool.tile([P, 1], f32, tag="inv_noise")
            nc.vector.reciprocal_approx_fast(inv_noise[:], noise[:])
            st.update(mag=mag, lvar=lvar, diff=diff, inv_noise=inv_noise)
            return st

        def emit_pass_b(st):
            ch = st["ch"]
            mag, lvar, diff = st["mag"], st["lvar"], st["diff"]
            inv_noise = st["inv_noise"]
            # dd = max(lvar/noise, 1)   (f32 out: reciprocal seed needs f32 in)
            dd = tpool.tile([P, NW, H], f32, tag="dd")
            nc.vector.tensor_scalar(
                dd[:], lvar[:], inv_noise[:], 1.0, Alu.mult, Alu.max
            )
            # rr = 1/dd (dd >= 1 so the approx-fast edge cases can't occur);
            # fp16 output via the write-stage cast
            from concourse.dve_ops import (
                RECIP_APPROX_FAST_CONSTS as _RC,
                RECIPROCAL_APPROX_FAST as _RF,
            )
            rr = tpool.tile([P, NW, H], f16, tag="rr")
            nc.vector._custom_dve(
                _RF, out=rr[:], in0=dd[:],
                s0=_RC["s0"], s1=_RC["s1"], imm2=_RC["imm2"],
            )
            # w1 = rr * diff  on GPSIMD (idle engine)
            w1 = tpool.tile([P, NW, H], f16, tag="w1")
            nc.gpsimd.tensor_tensor(w1[:], rr[:], diff[:], Alu.mult)
            # out = mag + w1  (4x STT)
            out_t = opool.tile([P, NW, H], f16, tag="out")
            nc.vector.scalar_tensor_tensor(
                out_t[:], in0=mag[:, :, 2:H + 2], scalar=1.0, in1=w1[:],
                op0=Alu.mult, op1=Alu.add,
            )
            nc.scalar.dma_start(
                y_out[ch].rearrange("(j p) h -> p j h", p=P), out_t[:]
            )

        prev = None
        for ch in range(NCH):
            st = emit_pass_a(ch)
            if prev is not None:
                emit_pass_b(prev)
            prev = st
        emit_pass_b(prev)

    nc.finalize()
    return nc


def _get_prog():
    if "prog" not in _PROG_CACHE:
        a, M, E, bx, bxe = _constants()
        _PROG_CACHE["a"] = a
        _PROG_CACHE["cm"] = M
        _PROG_CACHE["ce"] = E
        _PROG_CACHE["bx"] = bx
        _PROG_CACHE["bxe"] = bxe
        _PROG_CACHE["prog"] = _build_program()
    return _PROG_CACHE["prog"]


def _run(image, **spmd_kwargs):
    from concourse.bass_utils import run_bass_kernel_spmd

    nc = _get_prog()
    a = _PROG_CACHE["a"]
    # host prep: transpose to (b,c,w,h), scale by a[h], cast fp16
    xt = np.transpose(np.asarray(image, np.float64), (0, 1, 3, 2)) * a[None, None, None, :]
    xt16 = np.ascontiguousarray(xt.astype(np.float16)).reshape(NCORES, NCH, W, H)
    # halo rows per chunk: src rows (i*128 + {-8..-1, 128..135}) mod 512
    src = np.concatenate([np.arange(-8, 0), np.arange(128, 136)])  # (16,)
    rows = (np.arange(NW)[None, :] * P + src[:, None]) % W          # (16, NW)
    xh = np.ascontiguousarray(xt16[:, :, rows, :])                  # (8, NCH, 16, NW, H)
    consts = {k: _PROG_CACHE[k] for k in ("cm", "ce", "bx", "bxe")}
    consts["ones"] = np.ones((P, 1), np.float32)
    consts["onesr"] = np.ones((1, P), np.float32)
    in_maps = [{"x": xt16[c], "xh": xh[c], **consts} for c in range(NCORES)]
    res = run_bass_kernel_spmd(nc, in_maps, list(range(NCORES)), **spmd_kwargs)
    ys = np.stack([res.results[c]["y"] for c in range(NCORES)])  # (8, 12, W, H) f16
    out = ys.reshape(B, C, W, H).transpose(0, 1, 3, 2).astype(np.float32)
    return np.ascontiguousarray(out), res


def kernel(image):
    out, _ = _run(image)
    return out


# revision 5
# speedup vs baseline: 1.2114x; 1.2114x over previous
"""Trainium2 Bass kernel for nn_DenoiseModule (diffraction removal + 2x2 Wiener).

Math reduction (derived from the reference):
  - The reference FFT2 acts on the (W, C) axes of the (B,H,W,C)-transposed
    image. The Gaussian mask factorizes mask[h,w] = a[h]*s[w] and is constant
    along the channel-frequency axis, so the C-axis FFT cancels exactly.
  - Net effect: per (b,c,h) row, a 1-D circular convolution along W with the
    short kernel K = IFFT(s), scaled by a[h], then abs(), then the
    scipy-style 2x2 Wiener filter.
  - K's imaginary part contributes < 1e-5 rel error to the final output
    (it exists only because linspace(-256,256,512) is slightly asymmetric),
    so only the real part is convolved and |y| = abs(real conv).
  - The x255/255 scaling cancels; a[h] commutes with the W-conv so it is
    applied to the input on the host.

Device layout: batch-parallel over 8 cores (4 images each = 12 channels).
Per channel, data is W-on-partitions (host pre-transposes). Everything is
fp16 on device (the harness tolerance is 2e-2; fp16 keeps rel err ~1e-3):
  - conv: 2 fp16 matmuls per 128-row chunk (main band block + one 16-row
    halo matmul covering both cross-chunk edge triangles)
  - box sums: h-pairs pre-added on DVE (4x-mode STT ops), then 2 matmuls
    per chunk per box (in-chunk band + cross-chunk edge)
  - Wiener tail: 4x/2x DVE ops; the elementwise divide runs on GPSIMD.
"""
import numpy as np

B, C, H, W = 32, 3, 512, 512
NCORES = 8
BL = B // NCORES          # images per core
NCH = BL * C              # channels per core
P = 128
NW = W // P               # w-chunks
TAP = 8                   # conv band half-width
HALO = 16                 # halo rows per chunk (8 below + 8 above)
DR = 40.0


def _constants():
    x_lin = np.linspace(-256, 256, 512).astype(np.float64)
    g = np.exp(-(x_lin ** 2) / (2 * DR * DR))
    sh = (np.arange(512) + 256) % 512
    a = g[sh]                      # per-h scale (fft-order coords)
    s = g[sh]                      # per-kw mask
    K = np.fft.ifft(s)
    d = np.minimum(np.arange(512), 512 - np.arange(512))
    Kr = np.where(d <= TAP, np.real(K), 0.0)
    # main conv block (same for every chunk): M[p, c] = Kr[c - p], |c-p|<=TAP
    pp = np.arange(P)[:, None]
    cc = np.arange(P)[None, :]
    M = np.where(np.abs(cc - pp) <= TAP, Kr[(cc - pp) % 512], 0.0)
    # halo block: rows 0..7 are inputs at chunk_start-8+r, rows 8..15 are
    # inputs at chunk_start+128+(r-8); E[r, c] = Kr[c - src(r)]
    src = np.concatenate([np.arange(-8, 0), np.arange(128, 136)])
    rr_ = src[:, None]
    E = np.where(np.abs(cc - rr_) <= TAP, Kr[(cc - rr_) % 512], 0.0)
    # box lhsT: out[wo] = 0.25*(in[wo] + in[wo-1])
    bx = np.zeros((P, P))
    np.fill_diagonal(bx, 0.25)
    bx[np.arange(P - 1), np.arange(1, P)] = 0.25
    # edge block: out[0] += 0.25*in_prev[127]
    bxe = np.zeros((P, P))
    bxe[P - 1, 0] = 0.25
    f16 = np.float16
    return (a, M.astype(f16), E.astype(f16), bx.astype(f16), bxe.astype(f16))


_PROG_CACHE = {}


def _build_program():
    from contextlib import ExitStack
    import concourse.bacc as bacc
    import concourse.tile as tile
    from concourse import mybir

    f32 = mybir.dt.float32
    f16 = mybir.dt.float16
    Alu = mybir.AluOpType
    Act = mybir.ActivationFunctionType

    nc = bacc.Bacc(None)
    x_in = nc.declare_dram_parameter("x", [NCH, W, H], f16, isOutput=False)
    xh_in = nc.declare_dram_parameter("xh", [NCH, HALO, NW, H], f16, isOutput=False)
    cm_in = nc.declare_dram_parameter("cm", [P, P], f16, isOutput=False)
    ce_in = nc.declare_dram_parameter("ce", [HALO, P], f16, isOutput=False)
    bx_in = nc.declare_dram_parameter("bx", [P, P], f16, isOutput=False)
    bxe_in = nc.declare_dram_parameter("bxe", [P, P], f16, isOutput=False)
    ones_in = nc.declare_dram_parameter("ones", [P, 1], f32, isOutput=False)
    onesr_in = nc.declare_dram_parameter("onesr", [1, P], f32, isOutput=False)
    y_out = nc.declare_dram_parameter("y", [NCH, W, H], f16, isOutput=True)

    HP = H + 2

    with tile.TileContext(nc) as tc, ExitStack() as ctx:
        cpool = ctx.enter_context(tc.tile_pool(name="consts", bufs=1))
        cm_t = cpool.tile([P, P], f16, tag="cm")
        nc.sync.dma_start(cm_t[:], cm_in[:])
        ce_t = cpool.tile([HALO, P], f16, tag="ce")
        nc.sync.dma_start(ce_t[:], ce_in[:])
        bx_t = cpool.tile([P, P], f16, tag="bx")
        nc.sync.dma_start(bx_t[:], bx_in[:])
        bxe_t = cpool.tile([P, P], f16, tag="bxe")
        nc.sync.dma_start(bxe_t[:], bxe_in[:])
        ones_t = cpool.tile([P, 1], f32, tag="ones")
        nc.sync.dma_start(ones_t[:], ones_in[:])
        onesr_t = cpool.tile([1, P], f32, tag="onesr")
        nc.sync.dma_start(onesr_t[:], onesr_in[:])

        xpool = ctx.enter_context(tc.tile_pool(name="xin", bufs=3))
        hpool = ctx.enter_context(tc.tile_pool(name="halo", bufs=3))
        mpool = ctx.enter_context(tc.tile_pool(name="magp", bufs=3))
        spool = ctx.enter_context(tc.tile_pool(name="sqp", bufs=3))
        upool = ctx.enter_context(tc.tile_pool(name="up", bufs=3))
        lpool = ctx.enter_context(tc.tile_pool(name="lvp", bufs=2))
        tpool = ctx.enter_context(tc.tile_pool(name="tmp", bufs=3))
        npool = ctx.enter_context(tc.tile_pool(name="noise", bufs=3))
        opool = ctx.enter_context(tc.tile_pool(name="outp", bufs=2))
        psum = ctx.enter_context(tc.tile_pool(name="ps", bufs=2, space="PSUM"))

        def emit_pass_a(ch):
            st = {"ch": ch}
            xin = xpool.tile([P, NW, H], f16, tag="xin")
            nc.sync.dma_start(
                xin[:], x_in[ch].rearrange("(j p) h -> p j h", p=P)
            )
            halo = hpool.tile([HALO, NW, H], f16, tag="halo")
            nc.sync.dma_start(halo[:], xh_in[ch])

            # mag/sq keep 2 leading zero cols per chunk: data at [2:H+2);
            # the h-1 shifted read [1:H+1) then sees a zero at h=0.
            mag = mpool.tile([P, NW, HP], f16, tag="mag")
            nc.vector.memset(mag[:, :, 0:2], 0.0)
            sq = spool.tile([P, NW, HP], f16, tag="sq")
            nc.vector.memset(sq[:, :, 0:2], 0.0)
            lvar = lpool.tile([P, NW, H], f16, tag="lvar")
            diff = lpool.tile([P, NW, H], f16, tag="diff")
            part = tpool.tile([P, NW], f32, tag="part")
            um_prev = None
            us_prev = None

            for i in range(NW):
                ps_y = psum.tile([P, H], f32, tag="ps_y")
                nc.tensor.matmul(ps_y[:], cm_t[:], xin[:, i, :],
                                 start=True, stop=False)
                nc.tensor.matmul(ps_y[:], ce_t[:], halo[:, i, :],
                                 start=False, stop=True)
                nc.scalar.activation(mag[:, i, 2:H + 2], ps_y[:], Act.Abs)
                # sq = y^2 on ACT straight from PSUM (second read of ps_y)
                nc.scalar.square(sq[:, i, 2:H + 2], ps_y[:])
                # h-pair pre-adds (plain TT, fp16 2x-1port candidates)
                um = upool.tile([P, H], f16, tag="um")
                nc.vector.tensor_tensor(
                    um[:], mag[:, i, 2:H + 2], mag[:, i, 1:H + 1], Alu.add
                )
                us = upool.tile([P, H], f16, tag="us")
                nc.vector.tensor_tensor(
                    us[:], sq[:, i, 2:H + 2], sq[:, i, 1:H + 1], Alu.add
                )
                # box sums on PE: ps_lm = box(mag)/4, ps_bs = box(sq)/4
                ps_lm = psum.tile([P, H], f32, tag="ps_lm")
                nc.tensor.matmul(ps_lm[:], bx_t[:], um[:],
                                 start=True, stop=(i == 0))
                if i > 0:
                    nc.tensor.matmul(ps_lm[:], bxe_t[:], um_prev[:],
                                     start=False, stop=True)
                ps_bs = psum.tile([P, H], f32, tag="ps_bs")
                nc.tensor.matmul(ps_bs[:], bx_t[:], us[:],
                                 start=True, stop=(i == 0))
                if i > 0:
                    nc.tensor.matmul(ps_bs[:], bxe_t[:], us_prev[:],
                                     start=False, stop=True)
                um_prev, us_prev = um, us

                lm2 = tpool.tile([P, H], f32, tag="lm2")
                nc.scalar.square(lm2[:], ps_lm[:])
                # lvar = ps_bs - lm2 ; accum -> per-partition rowsum for noise
                nc.vector.scalar_tensor_tensor(
                    lvar[:, i, :], in0=lm2[:], scalar=-1.0, in1=ps_bs[:],
                    op0=Alu.mult, op1=Alu.add, accum_out=part[:, i:i + 1],
                )
                # diff = lM - mag
                nc.vector.tensor_tensor(diff[:, i, :], ps_lm[:],
                                        mag[:, i, 2:H + 2], Alu.subtract)

            # ---- noise scalar (PE reduce + broadcast) ----
            pr = tpool.tile([P, 1], f32, tag="pr")
            nc.vector.tensor_reduce(pr[:], part[:], mybir.AxisListType.X, Alu.add)
            ps_n1 = psum.tile([P, H], f32, tag="ps_y")
            nc.tensor.matmul(ps_n1[:1, :1], ones_t[:], pr[:], start=True, stop=True)
            nb = npool.tile([1, 1], f32, tag="nb")
            nc.scalar.copy(nb[:], ps_n1[:1, :1])
            ps_n2 = psum.tile([P, H], f32, tag="ps_lm")
            nc.tensor.matmul(ps_n2[:, :1], onesr_t[:], nb[:],
                             start=True, stop=True)
            noise = npool.tile([P, 1], f32, tag="noise")
            nc.scalar.mul(noise[:], ps_n2[:, :1], 1.0 / (H * W))
            inv_noise = npool.tile([P, 1], f32, tag="inv_noise")
            nc.vector.reciprocal_approx_fast(inv_noise[:], noise[:])
            st.update(mag=mag, lvar=lvar, diff=diff, inv_noise=inv_noise)
            return st

        def emit_pass_b(st):
            ch = st["ch"]
            mag, lvar, diff = st["mag"], st["lvar"], st["diff"]
            inv_noise = st["inv_noise"]
            # dd = max(lvar/noise, 1)   (f32 out: reciprocal seed needs f32 in)
            dd = tpool.tile([P, NW, H], f32, tag="dd")
            nc.vector.tensor_scalar(
                dd[:], lvar[:], inv_noise[:], 1.0, Alu.mult, Alu.max
            )
            # rr = 1/dd (dd >= 1 so the approx-fast edge cases can't occur);
            # fp16 output via the write-stage cast
            from concourse.dve_ops import (
                RECIP_APPROX_FAST_CONSTS as _RC,
                RECIPROCAL_APPROX_FAST as _RF,
            )
            rr = tpool.tile([P, NW, H], f16, tag="rr")
            nc.vector._custom_dve(
                _RF, out=rr[:], in0=dd[:],
                s0=_RC["s0"], s1=_RC["s1"], imm2=_RC["imm2"],
            )
            # w1 = rr * diff  on GPSIMD (idle engine)
            w1 = tpool.tile([P, NW, H], f16, tag="w1")
            nc.gpsimd.tensor_tensor(w1[:], rr[:], diff[:], Alu.mult)
            # out = mag + w1  (plain TT, fp16)
            out_t = opool.tile([P, NW, H], f16, tag="out")
            nc.vector.tensor_tensor(
                out_t[:], mag[:, :, 2:H + 2], w1[:], Alu.add
            )
            nc.scalar.dma_start(
                y_out[ch].rearrange("(j p) h -> p j h", p=P), out_t[:]
            )

        prev = None
        for ch in range(NCH):
            st = emit_pass_a(ch)
            if prev is not None:
                emit_pass_b(prev)
            prev = st
        emit_pass_b(prev)

    nc.finalize()
    return nc


def _get_prog():
    if "prog" not in _PROG_CACHE:
        a, M, E, bx, bxe = _constants()
        _PROG_CACHE["a"] = a
        _PROG_CACHE["cm"] = M
        _PROG_CACHE["ce"] = E
        _PROG_CACHE["bx"] = bx
        _PROG_CACHE["bxe"] = bxe
        _PROG_CACHE["prog"] = _build_program()
    return _PROG_CACHE["prog"]


def _run(image, **spmd_kwargs):
    from concourse.bass_utils import run_bass_kernel_spmd

    nc = _get_prog()
    a = _PROG_CACHE["a"]
    # host prep: transpose to (b,c,w,h), scale by a[h], cast fp16
    xt = np.transpose(np.asarray(image, np.float64), (0, 1, 3, 2)) * a[None, None, None, :]
    xt16 = np.ascontiguousarray(xt.astype(np.float16)).reshape(NCORES, NCH, W, H)
    # halo rows per chunk: src rows (i*128 + {-8..-1, 128..135}) mod 512
    src = np.concatenate([np.arange(-8, 0), np.arange(128, 136)])  # (16,)
    rows = (np.arange(NW)[None, :] * P + src[:, None]) % W          # (16, NW)
    xh = np.ascontiguousarray(xt16[:, :, rows, :])                  # (8, NCH, 16, NW, H)
    consts = {k: _PROG_CACHE[k] for k in ("cm", "ce", "bx", "bxe")}
    consts["ones"] = np.ones((P, 1), np.float32)
    consts["onesr"] = np.ones((1, P), np.float32)
    in_maps = [{"x": xt16[c], "xh": xh[c], **consts} for c in range(NCORES)]
    res = run_bass_kernel_spmd(nc, in_maps, list(range(NCORES)), **spmd_kwargs)
    ys = np.stack([res.results[c]["y"] for c in range(NCORES)])  # (8, 12, W, H) f16
    out = ys.reshape(B, C, W, H).transpose(0, 1, 3, 2).astype(np.float32)
    return np.ascontiguousarray(out), res


def kernel(image):
    out, _ = _run(image)
    return out
